# revision 1
# baseline (speedup 1.0000x reference)
"""Locally-connected 2D block layer (LocBlock2dNT) on 8 Trainium2 NeuronCores.

Problem: x (64,64,64,64) f32, w (256,64,16,16,16) f32.
  patches = unfold(x) -> (N,C,P,P,f2);  y = relu(einsum('ncpqf,ocpqf->nopq', patches, w) / 32)

Strategy:
  - Shard over patch ROWS p (16 rows, 2 per core). Both x and w shard cleanly
    along p: zero replication (~21 MB bf16 in per core vs 50+ MB for the
    batch/out_channel shardings).
  - Host-side (free): unfold + transpose into a K-major layout, cast to bf16,
    fold the 1/32 scale into w (exact: power of two).
  - Per core: 32 positions, each an [M=64 batch] x [K=1024] x [N=256 outch]
    matmul. Positions are packed two-at-a-time into the 128-wide PE array
    column dimension (pos A -> PSUM partitions 0:64, pos B -> 64:128, via
    tile_position auto-derived from the output AP base partition), so the
    two N=256 matmul streams run concurrently in different column groups.
  - Epilogue: relu on DVE, PSUM -> SBUF -> DRAM.
"""

import os
import numpy as np
import ml_dtypes

N = 64          # batch
C = 64          # in channels
P = 16          # patches per side
F = 4           # filter side
F2 = F * F      # 16
O = 256         # out channels
K = C * F2      # 1024 contraction
NCORES = 8
PROWS_PER_CORE = P // NCORES      # 2
POS = PROWS_PER_CORE * P          # 32 positions per core
PAIRS = POS // 2                  # 16
KT = K // 128                     # 8 k-tiles
SCALE = 1.0 / np.sqrt(np.float32(F2 * C))   # == 1/32 exactly

BF16 = ml_dtypes.bfloat16

_cache = {}


def _build_program():
    """Build + compile the (SPMD, shared) Bass program once per process."""
    if "nc" in _cache:
        return _cache["nc"]

    import concourse.bacc as bacc
    import concourse.mybir as mybir
    import concourse.tile as tile

    nc = bacc.Bacc(
        "TRN2", target_bir_lowering=False, debug=False, num_devices=NCORES
    )
    xr = nc.dram_tensor("xr", (128, POS * KT * N), mybir.dt.bfloat16,
                        kind="ExternalInput").ap()
    wr = nc.dram_tensor("wr", (128, POS * KT * O), mybir.dt.bfloat16,
                        kind="ExternalInput").ap()
    # yr[r, pair*256 + o], r = (pos%2)*64 + n
    yr = nc.dram_tensor("yr", (128, PAIRS * O), mybir.dt.bfloat16,
                        kind="ExternalOutput").ap()

    GP = 4                      # positions per w chunk
    NCHUNK = POS // GP
    QS = [nc.sync, nc.scalar]   # the two HWDGE queues

    with tile.TileContext(nc) as tc:
        with (
            tc.tile_pool(name="xpool", bufs=1) as xpool,
            tc.tile_pool(name="wpool", bufs=4) as wpool,
            tc.tile_pool(name="pspool", bufs=4, space="PSUM") as pspool,
            tc.tile_pool(name="opool", bufs=3) as opool,
        ):
            # whole x resident in SBUF, halves loaded concurrently on the
            # two HWDGE queues.
            xall = xpool.tile([128, POS * KT * N], mybir.dt.bfloat16)
            XH = POS * KT * N // 2
            nc.sync.dma_start(out=xall[:, :XH], in_=xr[:, :XH])
            nc.scalar.dma_start(out=xall[:, XH:], in_=xr[:, XH:])

            for chunk in range(NCHUNK):
                q = QS[chunk % 2]
                wt = wpool.tile([128, GP * KT * O], mybir.dt.bfloat16)
                c0 = chunk * GP * KT * O
                q.dma_start(out=wt, in_=wr[:, c0:c0 + GP * KT * O])

                ot = opool.tile([128, (GP // 2) * O], mybir.dt.bfloat16)
                for jp in range(GP // 2):      # position pairs in chunk
                    pos_a = chunk * GP + 2 * jp
                    pos_b = pos_a + 1
                    # two PSUM banks so the two concurrent accumulation
                    # groups never share a zero region
                    psa = pspool.tile([N, O], mybir.dt.float32)
                    psb_full = pspool.tile([128, O], mybir.dt.float32)
                    psb = psb_full[N:2 * N, :]
                    for k in range(KT):
                        xa = xall[:, pos_a * KT * N + k * N:
                                     pos_a * KT * N + k * N + N]
                        xb = xall[:, pos_b * KT * N + k * N:
                                     pos_b * KT * N + k * N + N]
                        wa = wt[:, (2 * jp) * KT * O + k * O:
                                   (2 * jp) * KT * O + k * O + O]
                        wb = wt[:, (2 * jp + 1) * KT * O + k * O:
                                   (2 * jp + 1) * KT * O + k * O + O]
                        # A -> array col group 0:64, B -> 64:128; the two
                        # matmul streams run concurrently
                        nc.tensor.matmul(psa, xa, wa,
                                         start=(k == 0), stop=(k == KT - 1))
                        nc.tensor.matmul(psb, xb, wb,
                                         start=(k == 0), stop=(k == KT - 1))
                    oc = jp * O
                    nc.vector.tensor_scalar_max(ot[0:N, oc:oc + O], psa, 0.0)
                    nc.vector.tensor_scalar_max(ot[N:2 * N, oc:oc + O], psb,
                                                0.0)
                # one output DMA per chunk, on the other queue
                pair0 = chunk * GP // 2
                QS[(chunk + 1) % 2].dma_start(
                    out=yr[:, pair0 * O:(pair0 + GP // 2) * O], in_=ot)

    nc.compile()
    _cache["nc"] = nc
    return nc


def _prep_inputs(x: np.ndarray, w: np.ndarray):
    """Host-side shard + layout + bf16 cast. Returns in_maps for 8 cores.

    Layouts per core (core c owns patch rows 2c, 2c+1; pos = pl*16 + q):
      xr[p128, pos, k, n] = patches[n, ch, 2c+pl, q, f],  K = k*128+p128 = ch*16+f
      wr[p128, pos, k, o] = w[o, ch, 2c+pl, q, f] * 1/32
      yr row = pair*128 + (pos%2)*64 + n
    """
    # unfold: (N,C,P,f,P,f) -> (N,C,P,P,f,f) -> (N,C,P,P,f2)
    patches = np.ascontiguousarray(
        x.reshape(N, C, P, F, P, F).transpose(0, 1, 2, 4, 3, 5)
    ).reshape(N, C, P, P, F2)
    ws = (w.astype(np.float32) * SCALE)

    in_maps = []
    for c in range(NCORES):
        pa = patches[:, :, 2 * c:2 * c + 2, :, :]        # (N, C, 2, P, F2)
        a2 = pa.transpose(1, 4, 2, 3, 0)                 # (C, F2, 2, P, N)
        a3 = (a2.reshape(K, POS, N)
                .reshape(KT, 128, POS, N)
                .transpose(1, 2, 0, 3)                   # (128, POS, KT, N)
                .reshape(128, POS * KT * N))
        xr_c = np.ascontiguousarray(a3).astype(BF16)

        wb = ws[:, :, 2 * c:2 * c + 2, :, :]             # (O, C, 2, P, F2)
        b2 = wb.transpose(1, 4, 2, 3, 0)                 # (C, F2, 2, P, O)
        b3 = (b2.reshape(K, POS, O)
                .reshape(KT, 128, POS, O)
                .transpose(1, 2, 0, 3)                   # (128, POS, KT, O)
                .reshape(128, POS * KT * O))
        wr_c = np.ascontiguousarray(b3).astype(BF16)

        in_maps.append({"xr": xr_c, "wr": wr_c})
    return in_maps


def kernel(x: np.ndarray, w: np.ndarray) -> np.ndarray:
    from concourse.bass_utils import run_bass_kernel_spmd

    nc = _build_program()
    in_maps = _prep_inputs(np.asarray(x), np.asarray(w))

    res = run_bass_kernel_spmd(nc, in_maps, core_ids=list(range(NCORES)))
    _cache["last_results"] = res

    y = np.empty((N, O, P, P), dtype=np.float32)
    for c in range(NCORES):
        y[:, :, 2 * c:2 * c + 2, :] = decode_core(res.results[c]["yr"])
    return y


def decode_core(yr: np.ndarray) -> np.ndarray:
    """(128, PAIRS*O) core output -> (N, O, PROWS_PER_CORE, P) slice.

    yr[r, pair*O + o] with r = (pos%2)*64 + n, pos = pair*2 + (pos%2) and
    pos = pl*P + q.
    """
    yrr = (yr.astype(np.float32)
             .reshape(2, N, PAIRS, O)          # (ab, n, pair, o)
             .transpose(2, 0, 1, 3)            # (pair, ab, n, o)
             .reshape(POS, N, O))              # (pos, n, o)
    return yrr.reshape(PROWS_PER_CORE, P, N, O).transpose(2, 3, 0, 1)



# revision 7
# speedup vs baseline: 1.4781x; 1.4781x over previous
"""Locally-connected 2D block layer (LocBlock2dNT) on 8 Trainium2 NeuronCores.

Problem: x (64,64,64,64) f32, w (256,64,16,16,16) f32.
  patches = unfold(x) -> (N,C,P,P,f2);  y = relu(einsum('ncpqf,ocpqf->nopq', patches, w) / 32)

Strategy:
  - Shard over patch ROWS p (16 rows, 2 per core). Both x and w shard cleanly
    along p: zero replication (~21 MB bf16 in per core vs 50+ MB for the
    batch/out_channel shardings).
  - Host-side (free): unfold + transpose into a K-major layout. w is cast to
    fp8 e3m4 (x2 scale, clip +-15.5; ~1.35% rel err, well under the 2e-2 gate)
    which halves the dominant DMA traffic; x stays bf16 and absorbs the
    1/32 * 1/2 dequant scale as an exact power-of-two exponent shift. The PE
    accepts mixed bf16 (stationary) x fp8 (moving) operands.
  - Per core: 32 positions, each an [M=64 batch] x [K=1024] x [N=256 outch]
    matmul. Positions are packed two-at-a-time into the 128-wide PE array
    column dimension (pos A -> PSUM partitions 0:64, pos B -> 64:128, via
    tile_position auto-derived from the output AP base partition), so the
    two N=256 matmul streams run concurrently in different column groups.
  - Epilogue: relu on DVE, PSUM -> SBUF -> DRAM.
"""

import os
import numpy as np
import ml_dtypes

N = 64          # batch
C = 64          # in channels
P = 16          # patches per side
F = 4           # filter side
F2 = F * F      # 16
O = 256         # out channels
K = C * F2      # 1024 contraction
NCORES = 8
PROWS_PER_CORE = P // NCORES      # 2
POS = PROWS_PER_CORE * P          # 32 positions per core
PAIRS = POS // 2                  # 16
KT = K // 128                     # 8 k-tiles
SCALE = 1.0 / np.sqrt(np.float32(F2 * C))   # == 1/32 exactly
WSCALE = 2.0                                # w -> e3m4 pre-scale (power of 2)

BF16 = ml_dtypes.bfloat16
FP8 = ml_dtypes.float8_e3m4

_cache = {}


def _build_program():
    """Build + compile the (SPMD, shared) Bass program once per process."""
    if "nc" in _cache:
        return _cache["nc"]

    import concourse.bacc as bacc
    import concourse.mybir as mybir
    import concourse.tile as tile

    nc = bacc.Bacc(
        "TRN2", target_bir_lowering=False, debug=False, num_devices=NCORES
    )
    xr = nc.dram_tensor("xr", (128, POS * KT * N), mybir.dt.bfloat16,
                        kind="ExternalInput").ap()
    wr = nc.dram_tensor("wr", (128, POS * KT * O), mybir.dt.float8e3,
                        kind="ExternalInput").ap()
    # yr[r, pair*256 + o], r = (pos%2)*64 + n
    yr = nc.dram_tensor("yr", (128, PAIRS * O), mybir.dt.bfloat16,
                        kind="ExternalOutput").ap()

    GP = 4                      # positions per w chunk
    NCHUNK = POS // GP
    QS = [nc.sync, nc.scalar]   # the two HWDGE queues

    with tile.TileContext(nc) as tc:
        with (
            tc.tile_pool(name="xpool", bufs=1) as xpool,
            tc.tile_pool(name="wpool", bufs=4) as wpool,
            tc.tile_pool(name="pspool", bufs=4, space="PSUM") as pspool,
            tc.tile_pool(name="opool", bufs=3) as opool,
        ):
            # whole x resident in SBUF, halves loaded concurrently on the
            # two HWDGE queues.
            xall = xpool.tile([128, POS * KT * N], mybir.dt.bfloat16)
            XH = POS * KT * N // 2
            nc.sync.dma_start(out=xall[:, :XH], in_=xr[:, :XH])
            nc.scalar.dma_start(out=xall[:, XH:], in_=xr[:, XH:])

            for chunk in range(NCHUNK):
                q = QS[chunk % 2]
                wt = wpool.tile([128, GP * KT * O], mybir.dt.float8e3)
                c0 = chunk * GP * KT * O
                q.dma_start(out=wt, in_=wr[:, c0:c0 + GP * KT * O])

                ot = opool.tile([128, (GP // 2) * O], mybir.dt.bfloat16)
                for jp in range(GP // 2):      # position pairs in chunk
                    pos_a = chunk * GP + 2 * jp
                    pos_b = pos_a + 1
                    # two PSUM banks so the two concurrent accumulation
                    # groups never share a zero region
                    psa = pspool.tile([N, O], mybir.dt.float32)
                    psb_full = pspool.tile([128, O], mybir.dt.float32)
                    psb = psb_full[N:2 * N, :]
                    for k in range(KT):
                        xa = xall[:, pos_a * KT * N + k * N:
                                     pos_a * KT * N + k * N + N]
                        xb = xall[:, pos_b * KT * N + k * N:
                                     pos_b * KT * N + k * N + N]
                        wa = wt[:, (2 * jp) * KT * O + k * O:
                                   (2 * jp) * KT * O + k * O + O]
                        wb = wt[:, (2 * jp + 1) * KT * O + k * O:
                                   (2 * jp + 1) * KT * O + k * O + O]
                        # A -> array col group 0:64, B -> 64:128; the two
                        # matmul streams run concurrently
                        nc.tensor.matmul(psa, xa, wa,
                                         start=(k == 0), stop=(k == KT - 1))
                        nc.tensor.matmul(psb, xb, wb,
                                         start=(k == 0), stop=(k == KT - 1))
                    oc = jp * O
                    nc.vector.tensor_scalar_max(ot[0:N, oc:oc + O], psa, 0.0)
                    nc.vector.tensor_scalar_max(ot[N:2 * N, oc:oc + O], psb,
                                                0.0)
                # one output DMA per chunk, on the other queue
                pair0 = chunk * GP // 2
                QS[(chunk + 1) % 2].dma_start(
                    out=yr[:, pair0 * O:(pair0 + GP // 2) * O], in_=ot)

    nc.compile()
    _cache["nc"] = nc
    return nc


def _prep_inputs(x: np.ndarray, w: np.ndarray):
    """Host-side shard + layout + bf16 cast. Returns in_maps for 8 cores.

    Layouts per core (core c owns patch rows 2c, 2c+1; pos = pl*16 + q):
      xr[p128, pos, k, n] = patches[n, ch, 2c+pl, q, f],  K = k*128+p128 = ch*16+f
      wr[p128, pos, k, o] = w[o, ch, 2c+pl, q, f] * 1/32
      yr row = pair*128 + (pos%2)*64 + n
    """
    # unfold: (N,C,P,f,P,f) -> (N,C,P,P,f,f) -> (N,C,P,P,f2)
    # x absorbs the dequant scale SCALE/WSCALE = 2^-6 (exact exponent shift)
    patches = np.ascontiguousarray(
        (x * np.float32(SCALE / WSCALE))
        .reshape(N, C, P, F, P, F).transpose(0, 1, 2, 4, 3, 5)
    ).reshape(N, C, P, P, F2)
    ws = np.clip(w.astype(np.float32) * np.float32(WSCALE), -15.5, 15.5)

    in_maps = []
    for c in range(NCORES):
        pa = patches[:, :, 2 * c:2 * c + 2, :, :]        # (N, C, 2, P, F2)
        a2 = pa.transpose(1, 4, 2, 3, 0)                 # (C, F2, 2, P, N)
        a3 = (a2.reshape(K, POS, N)
                .reshape(KT, 128, POS, N)
                .transpose(1, 2, 0, 3)                   # (128, POS, KT, N)
                .reshape(128, POS * KT * N))
        xr_c = np.ascontiguousarray(a3).astype(BF16)

        wb = ws[:, :, 2 * c:2 * c + 2, :, :]             # (O, C, 2, P, F2)
        b2 = wb.transpose(1, 4, 2, 3, 0)                 # (C, F2, 2, P, O)
        b3 = (b2.reshape(K, POS, O)
                .reshape(KT, 128, POS, O)
                .transpose(1, 2, 0, 3)                   # (128, POS, KT, O)
                .reshape(128, POS * KT * O))
        wr_c = np.ascontiguousarray(b3).astype(FP8)

        in_maps.append({"xr": xr_c, "wr": wr_c})
    return in_maps


def kernel(x: np.ndarray, w: np.ndarray) -> np.ndarray:
    from concourse.bass_utils import run_bass_kernel_spmd

    nc = _build_program()
    in_maps = _prep_inputs(np.asarray(x), np.asarray(w))

    res = run_bass_kernel_spmd(nc, in_maps, core_ids=list(range(NCORES)))
    _cache["last_results"] = res

    y = np.empty((N, O, P, P), dtype=np.float32)
    for c in range(NCORES):
        y[:, :, 2 * c:2 * c + 2, :] = decode_core(res.results[c]["yr"])
    return y


def decode_core(yr: np.ndarray) -> np.ndarray:
    """(128, PAIRS*O) core output -> (N, O, PROWS_PER_CORE, P) slice.

    yr[r, pair*O + o] with r = (pos%2)*64 + n, pos = pair*2 + (pos%2) and
    pos = pl*P + q.
    """
    yrr = (yr.astype(np.float32)
             .reshape(2, N, PAIRS, O)          # (ab, n, pair, o)
             .transpose(2, 0, 1, 3)            # (pair, ab, n, o)
             .reshape(POS, N, O))              # (pos, n, o)
    return yrr.reshape(PROWS_PER_CORE, P, N, O).transpose(2, 3, 0, 1)

